# revision 8
# baseline (speedup 1.0000x reference)
"""TRN2 Bass kernel for nn_ConceptEmbeddingConceptPred.

Computes y = concat([einsum('bjd,ijd->bi', x, W_emb) + b_loo,
                     einsum('bjd,hjd->bh', x, W_full) + b_full], axis=1)
where W_emb is the leave-one-out scatter-embedding of W_loo (zero diagonal).

Flattened, this is a (4096 x 16384) @ (16384 x 136) GEMM.

Distribution: contraction(k)-parallel over the 8 cores — core c owns
concepts j in [16c, 16c+16) (k-slice of 2048). Each core computes a full
(4096, 136) partial product; partials are summed on the host (cheap:
8 x 1.1 MB), bias added, concatenated.

Per-core dataflow (bf16 operands, fp32 PSUM accumulation):
  - x is pre-transposed on the host to (k, b) layout and cast to bf16,
    so contraction sits on SBUF partitions with no on-device transposes
    and half the HBM traffic (16.8 MB/core).
  - The loo (128 cols) and full-probe (8 cols) weights concatenate on
    the *moving* side into one (128, 136) rhs per k-tile: each matmul
    uses an x tile as the 128x128 stationary and streams 136 weight
    columns, so every PE pass produces all 136 outputs for 128 batch
    rows (100% array utilization; 512 MMs/core = the PE roofline,
    ~59 ns/MM sustained warm).
  - x arrives in 5 batch-column chunks (1024,1024,1024,512,512), each
    chunk a contiguous DRAM block per k-tile so DMA lines are 2 KB/1 KB
    per partition (the two HWDGE queues sustain ~170-190 GB/s each at
    >=2 KB lines vs ~130 at 1 KB). Tapered tail chunks shrink the
    end-of-kernel serialization (last chunk's compute can only start
    once its whole per-tile transfer lands).
  - Outputs: per 128-batch group, 16 matmuls accumulate in one PSUM
    bank; DVE copies to SBUF as bf16; groups of 4 share one output DMA
    (amortizes the ~2 us SWDGE fixed cost); the last two batches ride
    the sync HWDGE queue, which is idle by then, to cut the tail.
"""

import sys

for _p in ("/opt/trn_rl_repo",):
    if _p not in sys.path:
        sys.path.append(_p)

import numpy as np
import ml_dtypes
import concourse.bacc as bacc
import concourse.mybir as mybir
import concourse.tile as tile
from concourse.bass_utils import run_bass_kernel_spmd

dt = mybir.dt
bf16 = ml_dtypes.bfloat16

B, C, D, H = 4096, 128, 128, 8
M = C + H  # 136 output cols
NCORES = 8
JPC = C // NCORES  # 16 concept (= k) tiles per core
KPC = JPC * D  # 2048 contraction elements per core
CHUNKS = (2048, 1024, 768, 256)  # batch-column chunks of the x stream
NG = B // 128  # 32 batch groups of 128
OBATCH = 4  # groups per output DMA

_nc_cache = None


def _build():
    global _nc_cache
    if _nc_cache is not None:
        return _nc_cache

    nc = bacc.Bacc(
        "TRN2", target_bir_lowering=False, debug=False, num_devices=NCORES
    )
    xc_d = [
        nc.dram_tensor(f"xc{c}", (JPC, 128, ck), dt.bfloat16, kind="ExternalInput").ap()
        for c, ck in enumerate(CHUNKS)
    ]
    wc_d = nc.dram_tensor(
        "wc", (D, JPC, M), dt.bfloat16, kind="ExternalInput"
    ).ap()
    y_d = nc.dram_tensor(
        "y_part", (NG, 128, M), dt.bfloat16, kind="ExternalOutput"
    ).ap()

    with tile.TileContext(nc) as tc:
        with (
            tc.tile_pool(name="wpool", bufs=1) as wpool,
            tc.tile_pool(name="xpool", bufs=1) as xpool,
            tc.tile_pool(name="ypool", bufs=3) as ypool,
            tc.tile_pool(name="psum", bufs=4, space="PSUM") as psum,
        ):
            wc = wpool.tile([D, JPC, M], dt.bfloat16)
            nc.sync.dma_start(wc[:], wc_d[:])

            xk = [
                xpool.tile([128, B], dt.bfloat16, name=f"xk{t}", tag=f"x{t}")
                for t in range(JPC)
            ]
            c0 = 0
            for c, ck in enumerate(CHUNKS):
                for t in range(JPC):
                    # gpsimd (SWDGE) is idle until outputs start ~25us in;
                    # let it carry a slice of the early big chunks
                    if c < 2 and t >= 14:
                        eng = nc.gpsimd
                    else:
                        eng = nc.sync if t % 2 == 0 else nc.scalar
                    eng.dma_start(xk[t][:, c0 : c0 + ck], xc_d[c][t])
                c0 += ck

            nbat = NG // OBATCH
            for ob in range(nbat):
                yb = ypool.tile([128, OBATCH, M], dt.bfloat16, tag="yb")
                for i in range(OBATCH):
                    g = ob * OBATCH + i
                    b0 = g * 128
                    acc = psum.tile([128, M], dt.float32, tag="acc")
                    for t in range(JPC):
                        nc.tensor.matmul(
                            acc[:],
                            xk[t][:, b0 : b0 + 128],
                            wc[:, t, :],
                            start=(t == 0),
                            stop=(t == JPC - 1),
                        )
                    nc.vector.tensor_copy(yb[:, i, :], acc[:])
                dst = y_d[ob * OBATCH : (ob + 1) * OBATCH].rearrange("f p m -> p f m")
                eng = nc.sync if ob >= nbat - 2 else nc.gpsimd
                eng.dma_start(dst, yb[:])

    nc.compile()
    _nc_cache = nc
    return nc


def _embed_loo_weights(W_loo):
    # probe i sees concepts j != i; scatter into (C, C, D) with zero row at j=i
    I = np.arange(C)[:, None]
    J = np.arange(C)[None, :]
    src = np.clip(J - (J > I).astype(np.int64), 0, C - 2)  # (C, C)
    W_emb = np.take_along_axis(W_loo, src[:, :, None], axis=1)  # (C, C, D)
    return W_emb * (J != I)[:, :, None].astype(W_loo.dtype)


def _prep_in_maps(x, W_loo, W_full):
    x = np.asarray(x, dtype=np.float32)
    W_emb = _embed_loo_weights(np.asarray(W_loo, dtype=np.float32))
    W_full = np.asarray(W_full, dtype=np.float32)
    xbf = x.reshape(B, C * D).astype(bf16)
    in_maps = []
    for c in range(NCORES):
        xt_c = np.ascontiguousarray(xbf[:, c * KPC : (c + 1) * KPC].T)  # (KPC, B)
        m = {}
        c0 = 0
        for ci, ck in enumerate(CHUNKS):
            m[f"xc{ci}"] = np.ascontiguousarray(
                xt_c[:, c0 : c0 + ck].reshape(JPC, 128, ck)
            )
            c0 += ck
        jsl = slice(c * JPC, (c + 1) * JPC)
        # rhs layout (d, t, i): loo output cols 0..127, full-probe 128..135
        wl_c = W_emb[:, jsl, :].transpose(2, 1, 0)  # (D, JPC, C)
        wf_c = W_full[:, jsl, :].transpose(2, 1, 0)  # (D, JPC, H)
        m["wc"] = np.ascontiguousarray(
            np.concatenate([wl_c, wf_c], axis=2).astype(bf16)
        )
        in_maps.append(m)
    return in_maps


def _assemble(results, b_loo, b_full):
    y = np.zeros((B, M), np.float64)
    for r in results:
        y += r["y_part"].reshape(B, M).astype(np.float64)
    bias = np.concatenate(
        [np.asarray(b_loo, np.float64), np.asarray(b_full, np.float64)]
    )
    return (y + bias[None, :]).astype(np.float32)


def run_spmd(x, W_loo, b_loo, W_full, b_full, trace=False):
    nc = _build()
    in_maps = _prep_in_maps(x, W_loo, W_full)
    res = run_bass_kernel_spmd(
        nc, in_maps, core_ids=list(range(NCORES)), trace=trace
    )
    return _assemble(res.results, b_loo, b_full), res


def kernel(x, W_loo, b_loo, W_full, b_full):
    out, _ = run_spmd(x, W_loo, b_loo, W_full, b_full)
    return out
